# revision 24
# baseline (speedup 1.0000x reference)
"""MoE LoRA linear layer kernel for Trainium2, data-parallel over 8 NeuronCores.

Math (per token n):
    down = h @ down_w.T                      [N, 64]
    mask[n, r] = val[n, k] if idx[n, k] == r else 0   (indices distinct per row)
    out = (down * mask) @ up_w.T             [N, 4096]

Sharding: tokens split 8 ways (2048/core); LoRA weights replicated.

The kernel is HBM-bound (h in + out out dominate), so the design goal is
pure streaming at DMA line rate with all compute hidden underneath:

  * h is pre-transposed and pre-packed ON HOST into the exact SBUF image
    the down-projection wants ([i-chunk partitions, token free dim]) so
    every load is one fat contiguous 1 MB DMA and the PE never spends
    cycles transposing h.
  * h and out travel as bf16 (host casts) -> DMA bytes halve: 16 MB in +
    16 MB out per core ~= 90 us at 358 GB/s/core HBM. PSUM accumulation
    stays fp32; measured rel err 5.6e-3 vs the 2e-2 gate.
  * the top-k scatter mask is materialized host-side (a layout transform
    of the idx/val tensors, [64, NT] bf16, 256 KB/core) and applied as a
    single elementwise multiply against the down-proj PSUM per tile.

Schedule: the PE must never idle long enough for the HAM activity
monitor to re-throttle it to 1.2 GHz (a >~3.4 us gap halves matmul
throughput until it re-warms, which cascaded to +35 us in flat
schedules). Three measures:

  * ~40 warmup matmuls on a memset tile fill the initial DMA wait so the
    PE hits the first real matmul already at 2.4 GHz.
  * software pipelining across token tiles: the 32 down-proj matmuls of
    tile tt+1 are woven (2 at a time) between the up-proj PSUM pairs of
    tile tt, so when an up-proj matmul would stall on a PSUM bank whose
    copy is still draining, the PE has independent down-proj work in its
    queue instead of idling.
  * stores are triggered from the (otherwise idle) GpSimd SWDGE so their
    semaphore waits never delay the next tile's load triggers on the
    Sync HWDGE ring; const loads ride the separate Scalar HWDGE ring.

Up-proj PSUM pairs: 2 matmuls fill a 2-bank [128, 1024] PSUM tile, then
one fat cast-copy (alternating DVE/ACT) evacuates it to the bf16 output
staging tile -> halves the per-instruction copy overhead vs per-bank
copies. PSUM budget: 3 pair bufs (6 banks) + 1 down + 1 warmup = 8 banks.
"""

import sys

for p in ("/opt/trn_rl_repo", "/opt/pypackages"):
    if p not in sys.path:
        sys.path.insert(0, p)

import numpy as np
import ml_dtypes

BF16 = ml_dtypes.bfloat16

N, D_IN, D_OUT, RANK, TOPK = 16384, 4096, 4096, 64, 8
NCORES = 8
NT = N // NCORES          # tokens per core = 2048
P = 128                   # partitions
TT = 512                  # token tile (down-matmul free dim = 1 PSUM bank)
NKC = D_IN // P           # 32 contraction chunks for down proj
NJ = TT // P              # 4 x 128-token chunks per tile
NTILES = NT // TT         # 4 token tiles per core
OT = 512                  # output col tile (1 PSUM bank)
NOT = D_OUT // OT         # 8 output col tiles
NQ = 4                    # load quarters per hT tile (1 MB each)
QK = NKC // NQ            # 8 contraction chunks per quarter
NWARM = 44                # PE warmup matmuls (cover the first DMA wait)

_CACHE = {}


def _build_program():
    import concourse.bacc as bacc
    import concourse.mybir as mybir
    from concourse import tile

    f32 = mybir.dt.float32
    bf16 = mybir.dt.bfloat16
    # Bacc (not plain Bass): its finalize() runs move_matmul_waits_to_-
    # ldweights + generate_event_semaphores, which split semaphore waits to
    # satisfy the TRN2 one-wait-per-instruction constraint.
    nc = bacc.Bacc()

    ht = nc.declare_dram_parameter("ht", [NTILES * NQ * P, QK * TT], bf16,
                                   isOutput=False)
    dwt = nc.declare_dram_parameter("dwt", [P, NKC * RANK], bf16,
                                    isOutput=False)
    upt = nc.declare_dram_parameter("upt", [RANK, D_OUT], bf16,
                                    isOutput=False)
    maskt = nc.declare_dram_parameter("maskt", [RANK, NT], bf16,
                                      isOutput=False)
    out = nc.declare_dram_parameter("out", [NT, D_OUT], bf16, isOutput=True)

    with tile.TileContext(nc) as tc:
        with (
            tc.tile_pool(name="const", bufs=1) as const,
            tc.tile_pool(name="hT", bufs=3) as hT_pool,
            tc.tile_pool(name="resT", bufs=2) as resT_pool,
            tc.tile_pool(name="outsb", bufs=4) as out_pool,
            tc.tile_pool(name="psum_dn", bufs=1, space="PSUM") as psum_dn_pool,
            tc.tile_pool(name="psum_up", bufs=3, space="PSUM") as psum_up_pool,
        ):
            dwt_sb = const.tile([P, NKC * RANK], bf16)
            upt_sb = const.tile([RANK, D_OUT], bf16)
            maskt_sb = const.tile([RANK, NT], bf16)
            # consts ride the Scalar HWDGE ring, h loads the Sync ring
            nc.scalar.dma_start(out=dwt_sb[:], in_=dwt[:, :])
            nc.scalar.dma_start(out=upt_sb[:], in_=upt[:, :])
            nc.scalar.dma_start(out=maskt_sb[:], in_=maskt[:, :])

            hT = [None] * NTILES
            psum_dn = [None] * NTILES
            resT = [None] * NTILES

            def emit_load(tt):
                hT[tt] = hT_pool.tile([P, NKC * TT], bf16, name="hT")
                for q in range(NQ):
                    row = (tt * NQ + q) * P
                    nc.sync.dma_start(
                        out=hT[tt][:, q * QK * TT:(q + 1) * QK * TT],
                        in_=ht[row:row + P, :],
                    )

            def emit_down_pair(tt, ki2):
                for ki in (2 * ki2, 2 * ki2 + 1):
                    nc.tensor.matmul(
                        psum_dn[tt][:],
                        lhsT=dwt_sb[:, ki * RANK:(ki + 1) * RANK],
                        rhs=hT[tt][:, ki * TT:(ki + 1) * TT],
                        start=(ki == 0),
                        stop=(ki == NKC - 1),
                    )

            def emit_mul(tt):
                resT[tt] = resT_pool.tile([RANK, TT], bf16, name="resT")
                nc.vector.tensor_mul(
                    resT[tt][:],
                    psum_dn[tt][:],
                    maskt_sb[:, tt * TT:(tt + 1) * TT],
                )

            # all load triggers upfront: nothing ever queues ahead of a
            # load on the Sync ring. load(3)'s triggers just park at the
            # queue head until hT buffer 0 frees (hT pool bufs=3).
            for _tt in range(NTILES):
                emit_load(_tt)

            # PE clock warmup: HAM un-throttles after ~3.4 us of sustained
            # matmul activity; these junk matmuls span the initial load
            # wait so real matmuls start at 2.4 GHz instead of 1.2.
            warm = const.tile([P, OT], bf16)
            nc.vector.memset(warm[:], 0.5)
            junk = psum_up_pool.tile([P, OT], f32, name="junk", bufs=1)
            for _ in range(NWARM):
                nc.tensor.matmul(
                    junk[:], lhsT=warm[:, :P], rhs=warm[:],
                    start=True, stop=True,
                )

            psum_dn[0] = psum_dn_pool.tile([RANK, TT], f32, name="psum_dn")
            for ki2 in range(NKC // 2):
                emit_down_pair(0, ki2)
            emit_mul(0)

            copy_engines = [nc.vector.tensor_copy, nc.scalar.copy]
            cp_i = 0

            for tt in range(NTILES):
                nxt = tt + 1
                if nxt < NTILES:
                    psum_dn[nxt] = psum_dn_pool.tile([RANK, TT], f32, name="psum_dn")

                for j in range(NJ):
                    jj = tt * NJ + j
                    out_sb = out_pool.tile([P, D_OUT], bf16)
                    for op in range(NOT // 2):
                        psum_up = psum_up_pool.tile([P, 2 * OT], f32)
                        for h2 in range(2):
                            o = op * 2 + h2
                            nc.tensor.matmul(
                                psum_up[:, h2 * OT:(h2 + 1) * OT],
                                lhsT=resT[tt][:, j * P:(j + 1) * P],
                                rhs=upt_sb[:, o * OT:(o + 1) * OT],
                                start=True,
                                stop=True,
                            )
                        cp = copy_engines[cp_i % 2]
                        cp_i += 1
                        cp(
                            out=out_sb[:, op * 2 * OT:(op + 1) * 2 * OT],
                            in_=psum_up[:],
                        )
                        # weave 2 down-proj matmuls of the next tile in
                        # after every up-proj pair: independent PE work
                        # while this pair's copy drains its banks. The last
                        # tile has no down-proj left, so weave a junk
                        # matmul instead - it keeps the PE activity monitor
                        # from re-throttling the clock for the real matmuls.
                        if nxt < NTILES:
                            emit_down_pair(nxt, j * (NOT // 2) + op)
                        elif j < NJ - 1:
                            # (not in the final chunk: there junk would
                            # only delay the last copies/stores)
                            nc.tensor.matmul(
                                junk[:], lhsT=warm[:, :P], rhs=warm[:],
                                start=True, stop=True,
                            )
                        # last two chunks: store each quarter as soon as its
                        # copy lands, alternating rings (earlier drain ->
                        # shorter tail)
                        if jj >= NTILES * NJ - 2:
                            qe = nc.gpsimd if op % 2 == 0 else nc.sync
                            cs = slice(op * 2 * OT, (op + 1) * 2 * OT)
                            qe.dma_start(
                                out=out[jj * P:(jj + 1) * P, cs],
                                in_=out_sb[:, cs],
                            )
                    if jj < NTILES * NJ - 2:
                        # first half of the chunks store via GpSimd SWDGE
                        # only; once every load trigger is long gone, the
                        # back half alternates onto the Sync HWDGE ring too
                        store_engine = (
                            nc.gpsimd if jj % 2 == 0 or jj < 8 else nc.sync
                        )
                        store_engine.dma_start(
                            out=out[jj * P:(jj + 1) * P, :],
                            in_=out_sb[:],
                        )
                if nxt < NTILES:
                    emit_mul(nxt)

    # Run the Bacc pipeline (register alloc + wait splitting for the TRN2
    # one-wait-per-instruction constraint) before the module is serialized.
    nc.finalize()
    return nc


def _get_program():
    if "nc" not in _CACHE:
        _CACHE["nc"] = _build_program()
    return _CACHE["nc"]


def prepare_in_maps(hidden_states, down_w, up_w, top_k_values, top_k_indices):
    h = np.ascontiguousarray(hidden_states, dtype=np.float32)
    dw = np.ascontiguousarray(down_w, dtype=np.float32)
    uw = np.ascontiguousarray(up_w, dtype=np.float32)
    vals = np.ascontiguousarray(top_k_values, dtype=np.float32)
    idx = np.asarray(top_k_indices).astype(np.int64)

    # hT image, quarter-major so each 1 MB load is contiguous:
    # ht[c][(tt*NQ + q)*128 + p, kl*512 + n] = h[c*NT + tt*512 + n,
    #                                            (q*QK + kl)*128 + p]
    ht = (
        h.astype(BF16)
        .reshape(NCORES, NTILES, TT, NQ, QK, P)
        .transpose(0, 1, 3, 5, 4, 2)
        .reshape(NCORES, NTILES * NQ * P, QK * TT)
    )
    ht = np.ascontiguousarray(ht)

    # dwT[p, ki*64 + r] = dw[r, ki*128 + p]
    dwt = np.ascontiguousarray(
        dw.reshape(RANK, NKC, P).transpose(2, 1, 0).reshape(P, NKC * RANK)
    ).astype(BF16)
    upt = np.ascontiguousarray(uw.T).astype(BF16)  # [64, 4096]

    # dense routed mask [N, 64] -> per-core maskT [64, NT]
    mask = np.zeros((N, RANK), dtype=np.float32)
    mask[np.arange(N)[:, None], idx] = vals
    maskt_all = mask.astype(BF16)

    in_maps = []
    for c in range(NCORES):
        s = slice(c * NT, (c + 1) * NT)
        in_maps.append(
            {
                "ht": ht[c],
                "dwt": dwt,
                "upt": upt,
                "maskt": np.ascontiguousarray(maskt_all[s].T),
            }
        )
    return in_maps


def kernel(hidden_states, down_w, up_w, top_k_values, top_k_indices, **_kw):
    from concourse.bass_utils import run_bass_kernel_spmd

    nc = _get_program()
    in_maps = prepare_in_maps(
        hidden_states, down_w, up_w, top_k_values, top_k_indices
    )
    res = run_bass_kernel_spmd(nc, in_maps, core_ids=list(range(NCORES)))
    return np.concatenate(
        [r["out"].astype(np.float32) for r in res.results], axis=0
    )


# revision 28
# speedup vs baseline: 1.1755x; 1.1755x over previous
"""MoE LoRA linear layer kernel for Trainium2, data-parallel over 8 NeuronCores.

Math (per token n):
    down = h @ down_w.T                      [N, 64]
    mask[n, r] = val[n, k] if idx[n, k] == r else 0   (indices distinct per row)
    out = (down * mask) @ up_w.T             [N, 4096]

Sharding: tokens split 8 ways (2048/core); LoRA weights replicated.

The kernel is HBM-bound (h in + out out dominate), so the design goal is
pure streaming at DMA line rate with all compute hidden underneath:

  * h is pre-transposed and pre-packed ON HOST into the exact SBUF image
    the down-projection wants ([i-chunk partitions, token free dim]) so
    every load is one fat contiguous 1 MB DMA and the PE never spends
    cycles transposing h.
  * h and out travel as bf16 (host casts) -> DMA bytes halve: 16 MB in +
    16 MB out per core ~= 90 us at 358 GB/s/core HBM. PSUM accumulation
    stays fp32; measured rel err 5.6e-3 vs the 2e-2 gate.
  * the top-k scatter mask is materialized host-side (a layout transform
    of the idx/val tensors, [64, NT] bf16, 256 KB/core) and applied as a
    single elementwise multiply against the down-proj PSUM per tile.

Schedule: the PE must never idle long enough for the HAM activity
monitor to re-throttle it to 1.2 GHz (a >~3.4 us gap halves matmul
throughput until it re-warms, which cascaded to +35 us in flat
schedules). Three measures:

  * ~40 warmup matmuls on a memset tile fill the initial DMA wait so the
    PE hits the first real matmul already at 2.4 GHz.
  * software pipelining across token tiles: the 32 down-proj matmuls of
    tile tt+1 are woven (2 at a time) between the up-proj PSUM pairs of
    tile tt, so when an up-proj matmul would stall on a PSUM bank whose
    copy is still draining, the PE has independent down-proj work in its
    queue instead of idling.
  * stores are triggered from the (otherwise idle) GpSimd SWDGE so their
    semaphore waits never delay the next tile's load triggers on the
    Sync HWDGE ring; const loads ride the separate Scalar HWDGE ring.

Up-proj PSUM pairs: 2 matmuls fill a 2-bank [128, 1024] PSUM tile, then
one fat cast-copy (alternating DVE/ACT) evacuates it to the bf16 output
staging tile -> halves the per-instruction copy overhead vs per-bank
copies. PSUM budget: 3 pair bufs (6 banks) + 1 down + 1 warmup = 8 banks.
"""

import sys

for p in ("/opt/trn_rl_repo", "/opt/pypackages"):
    if p not in sys.path:
        sys.path.insert(0, p)

import numpy as np
import ml_dtypes

BF16 = ml_dtypes.bfloat16

N, D_IN, D_OUT, RANK, TOPK = 16384, 4096, 4096, 64, 8
NCORES = 8
NT = N // NCORES          # tokens per core = 2048
P = 128                   # partitions
TT = 512                  # token tile (down-matmul free dim = 1 PSUM bank)
NKC = D_IN // P           # 32 contraction chunks for down proj
NJ = TT // P              # 4 x 128-token chunks per tile
NTILES = NT // TT         # 4 token tiles per core
OT = 512                  # output col tile (1 PSUM bank)
NOT = D_OUT // OT         # 8 output col tiles
NQ = 4                    # load quarters per hT tile (1 MB each)
QK = NKC // NQ            # 8 contraction chunks per quarter
NWARM = 44                # PE warmup matmuls (cover the first DMA wait)

_CACHE = {}


def _build_program():
    import concourse.bacc as bacc
    import concourse.mybir as mybir
    from concourse import tile

    f32 = mybir.dt.float32
    bf16 = mybir.dt.bfloat16
    # Bacc (not plain Bass): its finalize() runs move_matmul_waits_to_-
    # ldweights + generate_event_semaphores, which split semaphore waits to
    # satisfy the TRN2 one-wait-per-instruction constraint.
    nc = bacc.Bacc()

    ht = nc.declare_dram_parameter("ht", [NTILES * NQ * P, QK * TT], bf16,
                                   isOutput=False)
    dwt = nc.declare_dram_parameter("dwt", [P, NKC * RANK], bf16,
                                    isOutput=False)
    upt = nc.declare_dram_parameter("upt", [RANK, D_OUT], bf16,
                                    isOutput=False)
    maskt = nc.declare_dram_parameter("maskt", [RANK, NT], bf16,
                                      isOutput=False)
    out = nc.declare_dram_parameter("out", [NT, D_OUT], bf16, isOutput=True)

    with tile.TileContext(nc) as tc:
        with (
            tc.tile_pool(name="const", bufs=1) as const,
            tc.tile_pool(name="hT", bufs=3) as hT_pool,
            tc.tile_pool(name="resT", bufs=2) as resT_pool,
            tc.tile_pool(name="outsb", bufs=4) as out_pool,
            tc.tile_pool(name="psum_dn", bufs=1, space="PSUM") as psum_dn_pool,
            tc.tile_pool(name="psum_up", bufs=3, space="PSUM") as psum_up_pool,
        ):
            dwt_sb = const.tile([P, NKC * RANK], bf16)
            upt_sb = const.tile([RANK, D_OUT], bf16)
            maskt_sb = const.tile([RANK, NT], bf16)
            # dwt rides the Scalar HWDGE ring (needed by the first down
            # matmuls ~10 us in); upt/maskt are deferred onto the Sync
            # ring behind tile 0's quarters (below) so they don't steal
            # early HBM bandwidth from the critical first loads - the
            # SDMA engines round-robin between rings at packet
            # granularity, and that ~2 us of theft is what pushed the
            # load(0) ramp gaps over the HAM re-throttle window on slow
            # runs. They still land ~20 us, well before first use ~28.
            nc.scalar.dma_start(out=dwt_sb[:], in_=dwt[:, :])

            hT = [None] * NTILES
            psum_dn = [None] * NTILES
            resT = [None] * NTILES

            def emit_load(tt):
                hT[tt] = hT_pool.tile([P, NKC * TT], bf16, name="hT")
                for q in range(NQ):
                    row = (tt * NQ + q) * P
                    nc.sync.dma_start(
                        out=hT[tt][:, q * QK * TT:(q + 1) * QK * TT],
                        in_=ht[row:row + P, :],
                    )

            def emit_down_pair(tt, ki2):
                for ki in (2 * ki2, 2 * ki2 + 1):
                    nc.tensor.matmul(
                        psum_dn[tt][:],
                        lhsT=dwt_sb[:, ki * RANK:(ki + 1) * RANK],
                        rhs=hT[tt][:, ki * TT:(ki + 1) * TT],
                        start=(ki == 0),
                        stop=(ki == NKC - 1),
                    )

            def emit_mul(tt):
                resT[tt] = resT_pool.tile([RANK, TT], bf16, name="resT")
                nc.vector.tensor_mul(
                    resT[tt][:],
                    psum_dn[tt][:],
                    maskt_sb[:, tt * TT:(tt + 1) * TT],
                )

            # all load triggers upfront: nothing ever queues ahead of a
            # load on the Sync ring. load(3)'s triggers just park at the
            # queue head until hT buffer 0 frees (hT pool bufs=3).
            emit_load(0)
            nc.sync.dma_start(out=upt_sb[:], in_=upt[:, :])
            nc.sync.dma_start(out=maskt_sb[:], in_=maskt[:, :])
            for _tt in range(1, NTILES):
                emit_load(_tt)

            # PE clock warmup: HAM un-throttles after ~3.4 us of sustained
            # matmul activity; these junk matmuls span the initial load
            # wait so real matmuls start at 2.4 GHz instead of 1.2.
            warm = const.tile([P, OT], bf16)
            nc.vector.memset(warm[:], 0.5)
            junk = psum_up_pool.tile([P, OT], f32, name="junk", bufs=1)
            for _ in range(NWARM):
                nc.tensor.matmul(
                    junk[:], lhsT=warm[:, :P], rhs=warm[:],
                    start=True, stop=True,
                )

            psum_dn[0] = psum_dn_pool.tile([RANK, TT], f32, name="psum_dn")
            for ki2 in range(NKC // 2):
                emit_down_pair(0, ki2)
            emit_mul(0)

            copy_engines = [nc.vector.tensor_copy, nc.scalar.copy]
            cp_i = 0

            for tt in range(NTILES):
                nxt = tt + 1
                if nxt < NTILES:
                    psum_dn[nxt] = psum_dn_pool.tile([RANK, TT], f32, name="psum_dn")

                for j in range(NJ):
                    jj = tt * NJ + j
                    out_sb = out_pool.tile([P, D_OUT], bf16)
                    for op in range(NOT // 2):
                        psum_up = psum_up_pool.tile([P, 2 * OT], f32)
                        for h2 in range(2):
                            o = op * 2 + h2
                            nc.tensor.matmul(
                                psum_up[:, h2 * OT:(h2 + 1) * OT],
                                lhsT=resT[tt][:, j * P:(j + 1) * P],
                                rhs=upt_sb[:, o * OT:(o + 1) * OT],
                                start=True,
                                stop=True,
                            )
                        cp = copy_engines[cp_i % 2]
                        cp_i += 1
                        cp(
                            out=out_sb[:, op * 2 * OT:(op + 1) * 2 * OT],
                            in_=psum_up[:],
                        )
                        # weave 2 down-proj matmuls of the next tile in
                        # after every up-proj pair: independent PE work
                        # while this pair's copy drains its banks. The last
                        # tile has no down-proj left, so weave a junk
                        # matmul instead - it keeps the PE activity monitor
                        # from re-throttling the clock for the real matmuls.
                        if nxt < NTILES:
                            emit_down_pair(nxt, j * (NOT // 2) + op)
                        else:
                            nc.tensor.matmul(
                                junk[:], lhsT=warm[:, :P], rhs=warm[:],
                                start=True, stop=True,
                            )
                        # last two chunks: store each quarter as soon as its
                        # copy lands, alternating rings (earlier drain ->
                        # shorter tail)
                        if jj >= NTILES * NJ - 2:
                            qe = nc.gpsimd if op % 2 == 0 else nc.sync
                            cs = slice(op * 2 * OT, (op + 1) * 2 * OT)
                            qe.dma_start(
                                out=out[jj * P:(jj + 1) * P, cs],
                                in_=out_sb[:, cs],
                            )
                    if jj < NTILES * NJ - 2:
                        # first half of the chunks store via GpSimd SWDGE
                        # only; once every load trigger is long gone, the
                        # back half alternates onto the Sync HWDGE ring too
                        store_engine = (
                            nc.gpsimd if jj % 2 == 0 or jj < 8 else nc.sync
                        )
                        store_engine.dma_start(
                            out=out[jj * P:(jj + 1) * P, :],
                            in_=out_sb[:],
                        )
                if nxt < NTILES:
                    emit_mul(nxt)

    # Run the Bacc pipeline (register alloc + wait splitting for the TRN2
    # one-wait-per-instruction constraint) before the module is serialized.
    nc.finalize()
    return nc


def _get_program():
    if "nc" not in _CACHE:
        _CACHE["nc"] = _build_program()
    return _CACHE["nc"]


def prepare_in_maps(hidden_states, down_w, up_w, top_k_values, top_k_indices):
    h = np.ascontiguousarray(hidden_states, dtype=np.float32)
    dw = np.ascontiguousarray(down_w, dtype=np.float32)
    uw = np.ascontiguousarray(up_w, dtype=np.float32)
    vals = np.ascontiguousarray(top_k_values, dtype=np.float32)
    idx = np.asarray(top_k_indices).astype(np.int64)

    # hT image, quarter-major so each 1 MB load is contiguous:
    # ht[c][(tt*NQ + q)*128 + p, kl*512 + n] = h[c*NT + tt*512 + n,
    #                                            (q*QK + kl)*128 + p]
    ht = (
        h.astype(BF16)
        .reshape(NCORES, NTILES, TT, NQ, QK, P)
        .transpose(0, 1, 3, 5, 4, 2)
        .reshape(NCORES, NTILES * NQ * P, QK * TT)
    )
    ht = np.ascontiguousarray(ht)

    # dwT[p, ki*64 + r] = dw[r, ki*128 + p]
    dwt = np.ascontiguousarray(
        dw.reshape(RANK, NKC, P).transpose(2, 1, 0).reshape(P, NKC * RANK)
    ).astype(BF16)
    upt = np.ascontiguousarray(uw.T).astype(BF16)  # [64, 4096]

    # dense routed mask [N, 64] -> per-core maskT [64, NT]
    mask = np.zeros((N, RANK), dtype=np.float32)
    mask[np.arange(N)[:, None], idx] = vals
    maskt_all = mask.astype(BF16)

    in_maps = []
    for c in range(NCORES):
        s = slice(c * NT, (c + 1) * NT)
        in_maps.append(
            {
                "ht": ht[c],
                "dwt": dwt,
                "upt": upt,
                "maskt": np.ascontiguousarray(maskt_all[s].T),
            }
        )
    return in_maps


def kernel(hidden_states, down_w, up_w, top_k_values, top_k_indices, **_kw):
    from concourse.bass_utils import run_bass_kernel_spmd

    nc = _get_program()
    in_maps = prepare_in_maps(
        hidden_states, down_w, up_w, top_k_values, top_k_indices
    )
    res = run_bass_kernel_spmd(nc, in_maps, core_ids=list(range(NCORES)))
    return np.concatenate(
        [r["out"].astype(np.float32) for r in res.results], axis=0
    )
